# revision 26
# baseline (speedup 1.0000x reference)
"""LocalWindowAttention Trainium2 kernel (Bass/Tile), 8-core SPMD.

Problem: x[B=4, S=4096, E=512] -> out[B, S, E]
  qkv = x @ W_qkv + b_qkv ; q,k,v = split(qkv)
  scores = (q @ k.T) / sqrt(E), banded mask |i-j| <= 64, softmax
  out = (attn @ v) @ W_out + b_out

Sharding: 8 cores = (batch b in 0..3) x (seq half h in 0..1). Each core owns
2048 query rows and loads a 64-row halo of x on each side (zero-padded at
sequence boundaries), computing q/k/v locally - no collectives.

Key structural choices:
  - W_out is folded into the v-projection on the host:
      (attn @ v) @ W_out = attn @ (x @ (W_v @ W_out))
    so the output projection disappears from the kernel. Since attention
    rows sum to 1, the output bias (b_v @ W_out + b_out) is folded into
    the v rows themselves (v'' = v' + b_vo added during the PSUM->SBUF
    copy), which makes softmax normalization a pure per-partition scale.
  - All matmul operands are bf16 (1 cycle/row at any moving size, FWL
    weight loads, half the DMA bytes). PSUM accumulation stays fp32.
  - Scores are computed TRANSPOSED, [key, query], with k-chunks as the
    stationary operand: the exp output is directly the stationary operand
    of the attended matmul -> no PE transposes at all.
  - The band mask is MULTIPLICATIVE (0/1 bf16) applied after exp as one
    DVE op per tile pair (raw scores are O(1) so unmasked exp is safe).
  - Row sums for softmax come from a ones-column appended to the v tiles
    (attended matmul emits [q, 256 feats + rowsum] per half); the final
    normalize is scalar-engine activation with per-partition scale 1/rowsum.
  - Inputs stream on two HW DMA queues (SP: xT + output, ACT: weights),
    ordered so the PE starts in ~5us and never starves.
"""

import sys

sys.path.insert(0, "/opt/trn_rl_repo")

import numpy as np
import ml_dtypes

import concourse.bass as bass  # noqa: F401  (registers types)
import concourse.tile as tile
from concourse import bacc, mybir
from concourse.bass_utils import run_bass_kernel_spmd

F32 = mybir.dt.float32
BF16 = mybir.dt.bfloat16
BF16_NP = ml_dtypes.bfloat16

B, S, E = 4, 4096, 512
WINDOW = 64
HALF = S // 2              # 2048 query rows per core
ROWS = HALF + 2 * WINDOW   # 2176 local rows incl. halo
NT = HALF // 128           # 16 query subtiles per core
NCH = ROWS // 128          # 17 v chunks

# qT matmul groups in xT col space (queries live at local rows [64, 2112)).
# Group boundaries align with the DMA chunk boundaries so each group's
# moving operand becomes ready as soon as its chunk lands.
QSLC = [(64, 192), (256, 256), (512, 320), (832, 448), (1280, 512),
        (1792, 320)]
# kT matmul groups (full local rows)
KSLC = [(0, 256), (256, 256), (512, 320), (832, 448), (1280, 512),
        (1792, 384)]

_NC_CACHE = {}


def _build():
    nc = bacc.Bacc("TRN2", target_bir_lowering=False, debug=False, num_devices=8)

    # e-interleaved layouts: element (p, e, c) = original row 128e+p, col c.
    # One dma_start then moves all four 128-row slices (descriptors spread
    # across all 16 HW DMA queues) instead of four serialized ~650ns issues.
    xT_d = nc.dram_tensor("xT", [128, 4, ROWS], BF16, kind="ExternalInput")
    wqkv_d = nc.dram_tensor("wqkv", [128, 4, 3 * E], BF16, kind="ExternalInput")
    bqk_d = nc.dram_tensor("bqk", [128, 8], F32, kind="ExternalInput")
    mask_d = nc.dram_tensor("masks", [128, 1536], BF16, kind="ExternalInput")
    brep_d = nc.dram_tensor("brep", [128, E], F32, kind="ExternalInput")
    out_d = nc.dram_tensor("out", [HALF, E], BF16, kind="ExternalOutput")

    ACT = mybir.ActivationFunctionType
    ALU = mybir.AluOpType

    with tile.TileContext(nc) as tc:
        with (
            tc.tile_pool(name="const", bufs=1) as const,
            tc.tile_pool(name="big", bufs=1) as big,
        ):
            # ---- constants ----
            wq_sb = const.tile([128, 4, 3 * E], BF16, name="wq", tag="wq")
            bqk_sb = const.tile([128, 8], F32, name="bqk", tag="bqk")
            mask_sb = const.tile([128, 12, 128], BF16, name="msk", tag="msk")
            brep_sb = const.tile([128, 2, 256], F32, name="brep", tag="brep")

            # ---- persistent products ----
            qT = [big.tile([128, HALF], BF16, name=f"qT{f}", tag=f"qT{f}")
                  for f in range(4)]
            kT = [big.tile([128, ROWS], BF16, name=f"kT{f}", tag=f"kT{f}")
                  for f in range(4)]
            # v rows with W_out and output bias folded in; per 128-row chunk:
            # [h, 257] where col 256 of each half is 1.0 (rowsum column)
            vaug = [big.tile([128, 2, 257], BF16, name=f"v{r}", tag=f"v{r}")
                    for r in range(NCH)]

            xTp = big.tile([128, 4, ROWS], BF16, name="xT", tag="xT")

            # warm tile zeroed first (gpsimd runs pre-barrier) so the PE
            # warmup isn't gated on the vector engine's late start
            warm = const.tile([128, 640], BF16, name="warm", tag="warm")
            nc.gpsimd.memset(warm[:, :], 0.0)
            # ones columns for the rowsum trick (off critical path)
            for r in range(NCH):
                nc.gpsimd.memset(vaug[r][:, :, 256:257], 1.0)

            # ---- input DMAs ----
            # Merged e-interleaved chunks: each issue covers all 4 e-slices
            # and its descriptors spread over the 16 HW queues. Transfers
            # drain roughly in issue order at ~360 GB/s aggregate, so chunks
            # are issued in exact PE-consumption order with small chunks
            # first: the PE starts as soon as wq[0:256]+xT[64:256] land and
            # never hits a >=1us idle gap (which would drop the clock to the
            # mid pstate for a 3.4us window).
            # Three DMA rings (sync/scalar/gpsimd), each only ~90 GB/s:
            # chunks are placed so every chunk's ring finishes it just
            # before the PE (MID clock until ~16us, then full) reaches the
            # group that consumes it.
            nc.sync.dma_start(out=xTp[:, :, 64:256], in_=xT_d[:, :, 64:256])
            nc.scalar.dma_start(out=wq_sb[:, :, 0:256],
                                in_=wqkv_d[:, :, 0:256])
            nc.gpsimd.dma_start(out=wq_sb[:, :, 256:E],
                                in_=wqkv_d[:, :, 256:E])
            nc.sync.dma_start(out=xTp[:, :, 256:512], in_=xT_d[:, :, 256:512])
            nc.scalar.dma_start(out=xTp[:, :, 512:832],
                                in_=xT_d[:, :, 512:832])
            nc.gpsimd.dma_start(out=xTp[:, :, 0:64], in_=xT_d[:, :, 0:64])
            nc.gpsimd.dma_start(out=xTp[:, :, 832:1280],
                                in_=xT_d[:, :, 832:1280])
            nc.sync.dma_start(out=wq_sb[:, :, E:E + 256],
                              in_=wqkv_d[:, :, E:E + 256])
            nc.scalar.dma_start(out=wq_sb[:, :, E + 256:2 * E],
                                in_=wqkv_d[:, :, E + 256:2 * E])
            nc.sync.dma_start(out=xTp[:, :, 1280:1792],
                              in_=xT_d[:, :, 1280:1792])
            nc.scalar.dma_start(out=bqk_sb, in_=bqk_d[:, :])
            nc.gpsimd.dma_start(out=xTp[:, :, 1792:ROWS],
                                in_=xT_d[:, :, 1792:ROWS])
            nc.scalar.dma_start(out=wq_sb[:, :, 2 * E:3 * E],
                                in_=wqkv_d[:, :, 2 * E:3 * E])
            nc.gpsimd.dma_start(out=mask_sb[:, :, :], in_=mask_d[:, :])
            nc.sync.dma_start(out=brep_sb[:, :, :], in_=brep_d[:, :])

            # (PE warmup matmuls are emitted at the top of the PSUM pool
            # block below; the warm tile is zeroed by gpsimd up top.)

            # ---- q/k projections + attention ----
            # One shared 6-bank PSUM pool for warmup/projections/scores/
            # attended plus a dedicated 2-bank pool for v chunks: pv's banks
            # never overlap pj's, so the first v matmul does not WAR-wait on
            # the last kT bias read (pool-transition gap).
            with (
                tc.tile_pool(name="pj", bufs=6, space="PSUM") as pj,
                tc.tile_pool(name="pv", bufs=2, space="PSUM") as pv,
                tc.tile_pool(name="attn", bufs=4) as attn,
            ):
                # PE warmup: 128-col matmuls deliberately do NOT trigger the
                # DVFS ramp to full clock -- the ~90 GB/s/ring DMA supply
                # cannot feed a full-speed PE before ~16us anyway, and an
                # underfed fast PE idles >1us and gets throttled. The MID
                # clock window rides out the DMA-bound phase; dense real
                # matmuls then ramp it. Sized to end as the first chunks
                # land (~12.5us).
                wps = pj.tile([128, 512], F32, name="wps", tag="pj")
                for _ in range(38):
                    nc.tensor.matmul(wps[:, 0:128], warm[:, 0:128],
                                     warm[:, 128:256], start=True, stop=True)

                def qT_group(si):
                    q0, qn = QSLC[si]
                    for f in range(4):
                        ps = pj.tile([128, 512], F32,
                                     name=f"pq{f}_{si}", tag="pj")
                        for e in range(4):
                            nc.tensor.matmul(
                                ps[:, :qn],
                                wq_sb[:, e, 128 * f:128 * (f + 1)],
                                xTp[:, e, q0:q0 + qn],
                                start=(e == 0), stop=(e == 3),
                            )
                        nc.vector.tensor_scalar_add(
                            qT[f][:, q0 - 64:q0 - 64 + qn], ps[:, :qn],
                            bqk_sb[:, f:f + 1],
                        )

                def kT_group(si):
                    k0, kn = KSLC[si]
                    for f in range(4):
                        ps = pj.tile([128, 512], F32,
                                     name=f"pk{f}_{si}", tag="pj")
                        for e in range(4):
                            nc.tensor.matmul(
                                ps[:, :kn],
                                wq_sb[:, e, E + 128 * f:E + 128 * (f + 1)],
                                xTp[:, e, k0:k0 + kn],
                                start=(e == 0), stop=(e == 3),
                            )
                        nc.scalar.activation(
                            out=kT[f][:, k0:k0 + kn], in_=ps[:, :kn],
                            func=ACT.Identity, bias=bqk_sb[:, 4 + f:5 + f],
                        )

                # group order tracks DMA chunk arrival: q-weights + early xT
                # chunks land first, k-weights a few us later, xT tail last
                qT_group(0)
                qT_group(1)
                qT_group(2)
                qT_group(3)
                kT_group(0)
                kT_group(1)
                qT_group(4)
                kT_group(2)
                kT_group(3)
                qT_group(5)
                kT_group(4)
                kT_group(5)

                # ---- attention tiles, processed in pairs ----
                # Tiles (2P, 2P+1) share key chunk 2P+1, so a pair needs 3
                # key chunks: c0=2P (queries 2P only), c1=2P+1 (both query
                # tiles, N=256), c2=2P+2 (queries 2P+1 only) -> 12 score
                # matmuls per pair instead of 16, one packed PSUM bank
                # [128,4,128], one exp and one mask op per pair.
                def v_chunks(r0, r1):
                    # v'' = x @ (W_v @ W_out) + (b_v @ W_out + b_out),
                    # natural [rows, feat] layout (bias fused into the copy)
                    for r in range(r0, r1):
                        ps = pv.tile([128, 2, 256], F32, name=f"pv{r}", tag="pv")
                        for e in range(4):
                            nc.tensor.matmul(
                                ps[:, :, :],
                                xTp[:, e, 128 * r:128 * (r + 1)],
                                wq_sb[:, e, 2 * E:3 * E],
                                start=(e == 0), stop=(e == 3),
                            )
                        nc.vector.tensor_add(
                            vaug[r][:, :, 0:256], ps[:, :, :], brep_sb[:, :, :])

                def pair(P):
                    t0, t1 = 2 * P, 2 * P + 1
                    ps4 = pj.tile([128, 4, 128], F32, name=f"s{P}", tag="pj")
                    for f in range(4):
                        nc.tensor.matmul(
                            ps4[:, 0:1, :],
                            kT[f][:, 128 * t0:128 * (t0 + 1)],
                            qT[f][:, 128 * t0:128 * (t0 + 1)],
                            start=(f == 0), stop=(f == 3),
                        )
                    for f in range(4):
                        nc.tensor.matmul(
                            ps4[:, 1:3, :],
                            kT[f][:, 128 * t1:128 * (t1 + 1)],
                            qT[f][:, 128 * t0:128 * (t0 + 2)],
                            start=(f == 0), stop=(f == 3),
                        )
                    for f in range(4):
                        nc.tensor.matmul(
                            ps4[:, 3:4, :],
                            kT[f][:, 128 * (t1 + 1):128 * (t1 + 2)],
                            qT[f][:, 128 * t1:128 * (t1 + 1)],
                            start=(f == 0), stop=(f == 3),
                        )
                    # exp (raw scores are O(1)), then one multiplicative 0/1
                    # band-mask op for the whole pair
                    mi = 1 if P == 0 else (2 if P == NT // 2 - 1 else 0)
                    ept = attn.tile([128, 4, 128], BF16, name=f"pe{P}", tag="pe")
                    if P == NT // 2 - 1:
                        # final pair: per-half exp/mask so the drain chain
                        # (nothing left to overlap with) is shorter
                        for hb in range(2):
                            nc.scalar.activation(
                                out=ept[:, 2 * hb:2 * hb + 2, :],
                                in_=ps4[:, 2 * hb:2 * hb + 2, :], func=ACT.Exp)
                            nc.vector.tensor_tensor(
                                ept[:, 2 * hb:2 * hb + 2, :],
                                ept[:, 2 * hb:2 * hb + 2, :],
                                mask_sb[:, 4 * mi + 2 * hb:4 * mi + 2 * hb + 2, :],
                                op=ALU.mult)
                    else:
                        nc.scalar.activation(
                            out=ept[:, :, :], in_=ps4[:, :, :], func=ACT.Exp)
                        nc.vector.tensor_tensor(
                            ept[:, :, :], ept[:, :, :],
                            mask_sb[:, 4 * mi:4 * mi + 4, :], op=ALU.mult)
                    # attended (unnormalized) + rowsum via the ones column
                    for ti, t in ((0, t0), (1, t1)):
                        final = (P == NT // 2 - 1 and ti == 1)
                        paA = pj.tile([128, 257], F32,
                                      name=f"paA{t}", tag="pj")
                        paB = pj.tile([128, 257], F32,
                                      name=f"paB{t}", tag="pj")
                        rd = attn.tile([128, 1], F32, name=f"rd{t}", tag="rd")
                        ost = attn.tile([128, 512], BF16,
                                        name=f"ost{t}", tag="ost")
                        if not final:
                            for kc in range(2):
                                eslc = ept[:, 2 * ti + kc:2 * ti + kc + 1, :]
                                nc.tensor.matmul(
                                    paA[:, :], eslc, vaug[t + kc][:, 0:1, :],
                                    start=(kc == 0), stop=(kc == 1),
                                )
                                nc.tensor.matmul(
                                    paB[:, :], eslc, vaug[t + kc][:, 1:2, :],
                                    start=(kc == 0), stop=(kc == 1),
                                )
                            nc.vector.reciprocal(rd[:], paA[:, 256:257])
                            # out = attended * (1/rowsum); bias in v rows
                            nc.scalar.activation(
                                out=ost[:, 0:256], in_=paA[:, 0:256],
                                func=ACT.Identity, scale=rd[:])
                            nc.vector.tensor_scalar_mul(
                                ost[:, 256:512], paB[:, 0:256], rd[:])
                            nc.sync.dma_start(
                                out=out_d[128 * t:128 * (t + 1), :],
                                in_=ost[:])
                        else:
                            # final tile: finish the A half (incl. rowsum)
                            # first and ship it while the B half's matmuls
                            # still run, halving the exposed drain chain
                            for kc in range(2):
                                nc.tensor.matmul(
                                    paA[:, :],
                                    ept[:, 2 * ti + kc:2 * ti + kc + 1, :],
                                    vaug[t + kc][:, 0:1, :],
                                    start=(kc == 0), stop=(kc == 1),
                                )
                            nc.vector.reciprocal(rd[:], paA[:, 256:257])
                            nc.scalar.activation(
                                out=ost[:, 0:256], in_=paA[:, 0:256],
                                func=ACT.Identity, scale=rd[:])
                            nc.sync.dma_start(
                                out=out_d[128 * t:128 * (t + 1), 0:256],
                                in_=ost[:, 0:256])
                            for kc in range(2):
                                nc.tensor.matmul(
                                    paB[:, :],
                                    ept[:, 2 * ti + kc:2 * ti + kc + 1, :],
                                    vaug[t + kc][:, 1:2, :],
                                    start=(kc == 0), stop=(kc == 1),
                                )
                            # split the trailing normalize+DMA into halves
                            # on alternating engines so the last exposed
                            # piece is only 128 cols
                            nc.vector.tensor_scalar_mul(
                                ost[:, 256:384], paB[:, 0:128], rd[:])
                            nc.scalar.dma_start(
                                out=out_d[128 * t:128 * (t + 1), 256:384],
                                in_=ost[:, 256:384])
                            nc.vector.tensor_scalar_mul(
                                ost[:, 384:512], paB[:, 128:256], rd[:])
                            nc.sync.dma_start(
                                out=out_d[128 * t:128 * (t + 1), 384:512],
                                in_=ost[:, 384:512])

                # v-chunk projections interleave with attention pairs: the
                # v matmuls fill the PE while each pair's exp/mask/normalize
                # chain runs on the other engines
                v_chunks(0, 3)
                pair(0)
                v_chunks(3, 5)
                pair(1)
                v_chunks(5, 7)
                pair(2)
                v_chunks(7, 9)
                pair(3)
                v_chunks(9, 11)
                pair(4)
                v_chunks(11, 13)
                pair(5)
                v_chunks(13, 15)
                pair(6)
                v_chunks(15, 17)
                pair(7)
    nc.compile()
    return nc


def _get_nc():
    if "nc" not in _NC_CACHE:
        _NC_CACHE["nc"] = _build()
    return _NC_CACHE["nc"]


def _prep_shared(W_qkv, b_qkv, W_out, b_out):
    scale = 1.0 / np.sqrt(np.float64(E))
    W = np.array(W_qkv, dtype=np.float64)
    Wo = np.array(W_out, dtype=np.float64)
    b = np.array(b_qkv, dtype=np.float64)
    bo = np.array(b_out, dtype=np.float64)

    wq = W[:, :E] * scale
    wk = W[:, E:2 * E]
    wvo = W[:, 2 * E:3 * E] @ Wo          # fold output projection into v
    wqkv = np.concatenate([wq, wk, wvo], axis=1)

    bq = b[:E] * scale
    bk = b[E:2 * E]
    bqk = np.stack([*(bq.reshape(4, 128)), *(bk.reshape(4, 128))], axis=1)
    bvo = b[2 * E:3 * E] @ Wo + bo        # folded output bias

    # e-interleave: [512, 1536] -> [128, 4, 1536] with (p, e, c) = row 128e+p
    wqkv_i = np.ascontiguousarray(
        wqkv.reshape(4, 128, 3 * E).transpose(1, 0, 2))
    shared = {
        "wqkv": wqkv_i.astype(np.float32).astype(BF16_NP),
        "bqk": np.ascontiguousarray(bqk.astype(np.float32)),
        "brep": np.ascontiguousarray(
            np.tile(bvo.astype(np.float32)[None, :], (128, 1))),
    }
    return shared


def _masks_for(h: int) -> np.ndarray:
    """Multiplicative 0/1 masks in TRANSPOSED [key-in-chunk, block, query]
    layout for PAIRED tiles. Blocks per pair: [c0 | c1(for t0) | c1(for t1)
    | c2]; c0/c1(t0) see the key chunk as window-low (upper-triangular),
    c1(t1)/c2 as window-high (lower-triangular). Variants along dim1:
    [interior | first-pair | last-pair]."""
    j = np.arange(128)[:, None]           # key index within chunk
    i = np.arange(128)[None, :]           # query index within tile
    ut = (j - i >= 0)                     # key chunk == query tile: jj-i in [0,128]
    lt = (j <= i)                         # key chunk one above: jj-i in [0,128]
    ut_e = ut & (j >= 64) if h == 0 else ut       # seq start: key >= 0
    lt_e = lt & (j < 64) if h == 1 else lt        # seq end: key < S
    interior = np.stack([ut, lt, ut, lt], axis=1)
    first = np.stack([ut_e, lt, ut, lt], axis=1)
    last = np.stack([ut, lt, ut, lt_e], axis=1)
    stacked = np.concatenate([interior, first, last], axis=1)  # [128, 12, 128]
    return np.ascontiguousarray(
        stacked.reshape(128, 1536).astype(np.float32)).astype(BF16_NP)


def _install_ntff_shim():
    """The agent image's antenv lacks axon_hooks; synthesize it from the
    boot module's ctypes NTFF driver so trace=True can capture HW timing."""
    import types
    if "antenv.axon_hooks" in sys.modules:
        return
    try:
        from trn_agent_boot.trn_boot import _ntff_profile_via_ctypes
        hook = _ntff_profile_via_ctypes("/opt/axon/libaxon_pjrt.so")
    except Exception:
        hook = None
    mod = types.ModuleType("antenv.axon_hooks")
    mod.get_axon_ntff_profile_hook = lambda: hook
    mod.set_axon_ntff_profile_hook = lambda h: None
    sys.modules["antenv.axon_hooks"] = mod
    # avoid S3 artifact upload attempts during local profile processing
    try:
        from concourse import bass_utils as _bu
        _bu.upload_artifacts = lambda tmpdir: tmpdir
    except Exception:
        pass


def kernel(x, W_qkv, b_qkv, W_out, b_out, _trace=False):
    x = np.asarray(x, dtype=np.float32)
    nc = _get_nc()
    shared = _prep_shared(W_qkv, b_qkv, W_out, b_out)
    masks = [_masks_for(0), _masks_for(1)]

    in_maps = []
    for core in range(8):
        b, h = divmod(core, 2)
        lo = h * HALF - WINDOW
        hi = lo + ROWS
        xh = np.zeros((ROWS, E), dtype=np.float32)
        s0, s1 = max(lo, 0), min(hi, S)
        xh[s0 - lo:s1 - lo] = x[b, s0:s1]
        # transpose then e-interleave: [512, ROWS] -> [128, 4, ROWS]
        xT_i = np.ascontiguousarray(
            xh.T.reshape(4, 128, ROWS).transpose(1, 0, 2))
        in_maps.append({
            "xT": xT_i.astype(BF16_NP),
            "masks": masks[h],
            **shared,
        })

    kwargs = {}
    if _trace:
        _install_ntff_shim()
        kwargs = dict(trace=True, trace_cores=[0])
    res = run_bass_kernel_spmd(nc, in_maps, core_ids=list(range(8)), **kwargs)

    out = np.empty((B, S, E), dtype=np.float32)
    for core in range(8):
        b, h = divmod(core, 2)
        out[b, h * HALF:(h + 1) * HALF] = res.results[core]["out"].astype(np.float32)
    if _trace:
        return out, res
    return out



# revision 28
# speedup vs baseline: 1.0751x; 1.0751x over previous
"""LocalWindowAttention Trainium2 kernel (Bass/Tile), 8-core SPMD.

Problem: x[B=4, S=4096, E=512] -> out[B, S, E]
  qkv = x @ W_qkv + b_qkv ; q,k,v = split(qkv)
  scores = (q @ k.T) / sqrt(E), banded mask |i-j| <= 64, softmax
  out = (attn @ v) @ W_out + b_out

Sharding: 8 cores = (batch b in 0..3) x (seq half h in 0..1). Each core owns
2048 query rows and loads a 64-row halo of x on each side (zero-padded at
sequence boundaries), computing q/k/v locally - no collectives.

Key structural choices:
  - W_out is folded into the v-projection on the host:
      (attn @ v) @ W_out = attn @ (x @ (W_v @ W_out))
    so the output projection disappears from the kernel. Since attention
    rows sum to 1, the output bias (b_v @ W_out + b_out) is folded into
    the v rows themselves (v'' = v' + b_vo added during the PSUM->SBUF
    copy), which makes softmax normalization a pure per-partition scale.
  - All matmul operands are bf16 (1 cycle/row at any moving size, FWL
    weight loads, half the DMA bytes). PSUM accumulation stays fp32.
  - Scores are computed TRANSPOSED, [key, query], with k-chunks as the
    stationary operand: the exp output is directly the stationary operand
    of the attended matmul -> no PE transposes at all.
  - The band mask is MULTIPLICATIVE (0/1 bf16) applied after exp as one
    DVE op per tile pair (raw scores are O(1) so unmasked exp is safe).
  - Row sums for softmax come from a ones-column appended to the v tiles
    (attended matmul emits [q, 256 feats + rowsum] per half); the final
    normalize is scalar-engine activation with per-partition scale 1/rowsum.
  - Inputs stream on two HW DMA queues (SP: xT + output, ACT: weights),
    ordered so the PE starts in ~5us and never starves.
"""

import sys

sys.path.insert(0, "/opt/trn_rl_repo")

import numpy as np
import ml_dtypes

import concourse.bass as bass  # noqa: F401  (registers types)
import concourse.tile as tile
from concourse import bacc, mybir
from concourse.bass_utils import run_bass_kernel_spmd

F32 = mybir.dt.float32
BF16 = mybir.dt.bfloat16
BF16_NP = ml_dtypes.bfloat16

B, S, E = 4, 4096, 512
WINDOW = 64
HALF = S // 2              # 2048 query rows per core
ROWS = HALF + 2 * WINDOW   # 2176 local rows incl. halo
NT = HALF // 128           # 16 query subtiles per core
NCH = ROWS // 128          # 17 v chunks

# qT matmul groups in xT col space (queries live at local rows [64, 2112)).
# Group boundaries align with the DMA chunk boundaries so each group's
# moving operand becomes ready as soon as its chunk lands.
QSLC = [(64, 192), (256, 256), (512, 320), (832, 448), (1280, 512),
        (1792, 320)]
# kT matmul groups (full local rows)
KSLC = [(0, 256), (256, 256), (512, 320), (832, 448), (1280, 512),
        (1792, 384)]

_NC_CACHE = {}


def _build():
    nc = bacc.Bacc("TRN2", target_bir_lowering=False, debug=False, num_devices=8)

    # e-interleaved layouts: element (p, e, c) = original row 128e+p, col c.
    # One dma_start then moves all four 128-row slices (descriptors spread
    # across all 16 HW DMA queues) instead of four serialized ~650ns issues.
    xT_d = nc.dram_tensor("xT", [128, 4, ROWS], BF16, kind="ExternalInput")
    wqkv_d = nc.dram_tensor("wqkv", [128, 4, 3 * E], BF16, kind="ExternalInput")
    bqk_d = nc.dram_tensor("bqk", [128, 8], F32, kind="ExternalInput")
    mask_d = nc.dram_tensor("masks", [128, 1536], BF16, kind="ExternalInput")
    brep_d = nc.dram_tensor("brep", [128, E], F32, kind="ExternalInput")
    out_d = nc.dram_tensor("out", [HALF, E], BF16, kind="ExternalOutput")

    ACT = mybir.ActivationFunctionType
    ALU = mybir.AluOpType

    with tile.TileContext(nc) as tc:
        with (
            tc.tile_pool(name="const", bufs=1) as const,
            tc.tile_pool(name="big", bufs=1) as big,
        ):
            # ---- constants ----
            wq_sb = const.tile([128, 4, 3 * E], BF16, name="wq", tag="wq")
            bqk_sb = const.tile([128, 8], F32, name="bqk", tag="bqk")
            mask_sb = const.tile([128, 12, 128], BF16, name="msk", tag="msk")
            brep_sb = const.tile([128, 2, 256], F32, name="brep", tag="brep")

            # ---- persistent products ----
            qT = [big.tile([128, HALF], BF16, name=f"qT{f}", tag=f"qT{f}")
                  for f in range(4)]
            kT = [big.tile([128, ROWS], BF16, name=f"kT{f}", tag=f"kT{f}")
                  for f in range(4)]
            # v rows with W_out and output bias folded in; per 128-row chunk:
            # [h, 257] where col 256 of each half is 1.0 (rowsum column)
            vaug = [big.tile([128, 2, 257], BF16, name=f"v{r}", tag=f"v{r}")
                    for r in range(NCH)]

            xTp = big.tile([128, 4, ROWS], BF16, name="xT", tag="xT")

            # warm tile zeroed first (gpsimd runs pre-barrier) so the PE
            # warmup isn't gated on the vector engine's late start
            warm = const.tile([128, 640], BF16, name="warm", tag="warm")
            nc.gpsimd.memset(warm[:, :], 0.0)
            # ones columns for the rowsum trick (off critical path)
            for r in range(NCH):
                nc.gpsimd.memset(vaug[r][:, :, 256:257], 1.0)

            # ---- input DMAs ----
            # Merged e-interleaved chunks: each issue covers all 4 e-slices
            # and its descriptors spread over the 16 HW queues. Transfers
            # drain roughly in issue order at ~360 GB/s aggregate, so chunks
            # are issued in exact PE-consumption order with small chunks
            # first: the PE starts as soon as wq[0:256]+xT[64:256] land and
            # never hits a >=1us idle gap (which would drop the clock to the
            # mid pstate for a 3.4us window).
            # DMA chunks are per-e column ranges [:, e, c0:c1]: each of the
            # 128 partition-rows is then ONE contiguous descriptor of
            # (c1-c0)*2 bytes. Keeping that >=1KB avoids the sub-512B
            # descriptor latency penalty that capped the merged-e layout at
            # ~160 GB/s aggregate. Consumption-ordered on two rings.
            for e in range(4):
                nc.sync.dma_start(out=xTp[:, e, 0:832],
                                  in_=xT_d[:, e, 0:832])
            for e in range(4):
                nc.scalar.dma_start(out=wq_sb[:, e, 0:E],
                                    in_=wqkv_d[:, e, 0:E])
            for e in range(4):
                nc.sync.dma_start(out=xTp[:, e, 832:ROWS],
                                  in_=xT_d[:, e, 832:ROWS])
            nc.scalar.dma_start(out=bqk_sb, in_=bqk_d[:, :])
            for e in range(4):
                nc.scalar.dma_start(out=wq_sb[:, e, E:2 * E],
                                    in_=wqkv_d[:, e, E:2 * E])
            for e in range(4):
                nc.gpsimd.dma_start(out=wq_sb[:, e, 2 * E:3 * E],
                                    in_=wqkv_d[:, e, 2 * E:3 * E])
            nc.scalar.dma_start(out=mask_sb[:, :, :], in_=mask_d[:, :])
            nc.gpsimd.dma_start(out=brep_sb[:, :, :], in_=brep_d[:, :])

            # (PE warmup matmuls are emitted at the top of the PSUM pool
            # block below; the warm tile is zeroed by gpsimd up top.)

            # ---- q/k projections + attention ----
            # One shared 6-bank PSUM pool for warmup/projections/scores/
            # attended plus a dedicated 2-bank pool for v chunks: pv's banks
            # never overlap pj's, so the first v matmul does not WAR-wait on
            # the last kT bias read (pool-transition gap).
            with (
                tc.tile_pool(name="pj", bufs=6, space="PSUM") as pj,
                tc.tile_pool(name="pv", bufs=2, space="PSUM") as pv,
                tc.tile_pool(name="attn", bufs=4) as attn,
            ):
                # PE warmup: 9 back-to-back 512-col matmuls ramp the DVFS
                # clock to full by ~11.2us, right as the first wq/xT chunks
                # land -- with big-descriptor DMA the supply can now feed a
                # full-speed PE from the start.
                wps = pj.tile([128, 512], F32, name="wps", tag="pj")
                for _ in range(9):
                    nc.tensor.matmul(wps[:, :], warm[:, 0:128],
                                     warm[:, 128:640], start=True, stop=True)

                def qT_group(si):
                    q0, qn = QSLC[si]
                    for f in range(4):
                        ps = pj.tile([128, 512], F32,
                                     name=f"pq{f}_{si}", tag="pj")
                        for e in range(4):
                            nc.tensor.matmul(
                                ps[:, :qn],
                                wq_sb[:, e, 128 * f:128 * (f + 1)],
                                xTp[:, e, q0:q0 + qn],
                                start=(e == 0), stop=(e == 3),
                            )
                        nc.vector.tensor_scalar_add(
                            qT[f][:, q0 - 64:q0 - 64 + qn], ps[:, :qn],
                            bqk_sb[:, f:f + 1],
                        )

                def kT_group(si):
                    k0, kn = KSLC[si]
                    for f in range(4):
                        ps = pj.tile([128, 512], F32,
                                     name=f"pk{f}_{si}", tag="pj")
                        for e in range(4):
                            nc.tensor.matmul(
                                ps[:, :kn],
                                wq_sb[:, e, E + 128 * f:E + 128 * (f + 1)],
                                xTp[:, e, k0:k0 + kn],
                                start=(e == 0), stop=(e == 3),
                            )
                        nc.scalar.activation(
                            out=kT[f][:, k0:k0 + kn], in_=ps[:, :kn],
                            func=ACT.Identity, bias=bqk_sb[:, 4 + f:5 + f],
                        )

                # group order tracks DMA chunk arrival: q-weights + early xT
                # chunks land first, k-weights a few us later, xT tail last
                qT_group(0)
                qT_group(1)
                qT_group(2)
                qT_group(3)
                kT_group(0)
                kT_group(1)
                qT_group(4)
                kT_group(2)
                kT_group(3)
                qT_group(5)
                kT_group(4)
                kT_group(5)

                # ---- attention tiles, processed in pairs ----
                # Tiles (2P, 2P+1) share key chunk 2P+1, so a pair needs 3
                # key chunks: c0=2P (queries 2P only), c1=2P+1 (both query
                # tiles, N=256), c2=2P+2 (queries 2P+1 only) -> 12 score
                # matmuls per pair instead of 16, one packed PSUM bank
                # [128,4,128], one exp and one mask op per pair.
                def v_chunks(r0, r1):
                    # v'' = x @ (W_v @ W_out) + (b_v @ W_out + b_out),
                    # natural [rows, feat] layout (bias fused into the copy)
                    for r in range(r0, r1):
                        ps = pv.tile([128, 2, 256], F32, name=f"pv{r}", tag="pv")
                        for e in range(4):
                            nc.tensor.matmul(
                                ps[:, :, :],
                                xTp[:, e, 128 * r:128 * (r + 1)],
                                wq_sb[:, e, 2 * E:3 * E],
                                start=(e == 0), stop=(e == 3),
                            )
                        nc.vector.tensor_add(
                            vaug[r][:, :, 0:256], ps[:, :, :], brep_sb[:, :, :])

                def pair(P):
                    t0, t1 = 2 * P, 2 * P + 1
                    ps4 = pj.tile([128, 4, 128], F32, name=f"s{P}", tag="pj")
                    for f in range(4):
                        nc.tensor.matmul(
                            ps4[:, 0:1, :],
                            kT[f][:, 128 * t0:128 * (t0 + 1)],
                            qT[f][:, 128 * t0:128 * (t0 + 1)],
                            start=(f == 0), stop=(f == 3),
                        )
                    for f in range(4):
                        nc.tensor.matmul(
                            ps4[:, 1:3, :],
                            kT[f][:, 128 * t1:128 * (t1 + 1)],
                            qT[f][:, 128 * t0:128 * (t0 + 2)],
                            start=(f == 0), stop=(f == 3),
                        )
                    for f in range(4):
                        nc.tensor.matmul(
                            ps4[:, 3:4, :],
                            kT[f][:, 128 * (t1 + 1):128 * (t1 + 2)],
                            qT[f][:, 128 * t1:128 * (t1 + 1)],
                            start=(f == 0), stop=(f == 3),
                        )
                    # exp (raw scores are O(1)), then one multiplicative 0/1
                    # band-mask op for the whole pair
                    mi = 1 if P == 0 else (2 if P == NT // 2 - 1 else 0)
                    ept = attn.tile([128, 4, 128], BF16, name=f"pe{P}", tag="pe")
                    if P == NT // 2 - 1:
                        # final pair: per-half exp/mask so the drain chain
                        # (nothing left to overlap with) is shorter
                        for hb in range(2):
                            nc.scalar.activation(
                                out=ept[:, 2 * hb:2 * hb + 2, :],
                                in_=ps4[:, 2 * hb:2 * hb + 2, :], func=ACT.Exp)
                            nc.vector.tensor_tensor(
                                ept[:, 2 * hb:2 * hb + 2, :],
                                ept[:, 2 * hb:2 * hb + 2, :],
                                mask_sb[:, 4 * mi + 2 * hb:4 * mi + 2 * hb + 2, :],
                                op=ALU.mult)
                    else:
                        nc.scalar.activation(
                            out=ept[:, :, :], in_=ps4[:, :, :], func=ACT.Exp)
                        nc.vector.tensor_tensor(
                            ept[:, :, :], ept[:, :, :],
                            mask_sb[:, 4 * mi:4 * mi + 4, :], op=ALU.mult)
                    # attended (unnormalized) + rowsum via the ones column
                    for ti, t in ((0, t0), (1, t1)):
                        final = (P == NT // 2 - 1 and ti == 1)
                        paA = pj.tile([128, 257], F32,
                                      name=f"paA{t}", tag="pj")
                        paB = pj.tile([128, 257], F32,
                                      name=f"paB{t}", tag="pj")
                        rd = attn.tile([128, 1], F32, name=f"rd{t}", tag="rd")
                        ost = attn.tile([128, 512], BF16,
                                        name=f"ost{t}", tag="ost")
                        if not final:
                            for kc in range(2):
                                eslc = ept[:, 2 * ti + kc:2 * ti + kc + 1, :]
                                nc.tensor.matmul(
                                    paA[:, :], eslc, vaug[t + kc][:, 0:1, :],
                                    start=(kc == 0), stop=(kc == 1),
                                )
                                nc.tensor.matmul(
                                    paB[:, :], eslc, vaug[t + kc][:, 1:2, :],
                                    start=(kc == 0), stop=(kc == 1),
                                )
                            nc.vector.reciprocal(rd[:], paA[:, 256:257])
                            # out = attended * (1/rowsum); bias in v rows
                            nc.scalar.activation(
                                out=ost[:, 0:256], in_=paA[:, 0:256],
                                func=ACT.Identity, scale=rd[:])
                            nc.vector.tensor_scalar_mul(
                                ost[:, 256:512], paB[:, 0:256], rd[:])
                            nc.sync.dma_start(
                                out=out_d[128 * t:128 * (t + 1), :],
                                in_=ost[:])
                        else:
                            # final tile: finish the A half (incl. rowsum)
                            # first and ship it while the B half's matmuls
                            # still run, halving the exposed drain chain
                            for kc in range(2):
                                nc.tensor.matmul(
                                    paA[:, :],
                                    ept[:, 2 * ti + kc:2 * ti + kc + 1, :],
                                    vaug[t + kc][:, 0:1, :],
                                    start=(kc == 0), stop=(kc == 1),
                                )
                            nc.vector.reciprocal(rd[:], paA[:, 256:257])
                            nc.scalar.activation(
                                out=ost[:, 0:256], in_=paA[:, 0:256],
                                func=ACT.Identity, scale=rd[:])
                            nc.sync.dma_start(
                                out=out_d[128 * t:128 * (t + 1), 0:256],
                                in_=ost[:, 0:256])
                            for kc in range(2):
                                nc.tensor.matmul(
                                    paB[:, :],
                                    ept[:, 2 * ti + kc:2 * ti + kc + 1, :],
                                    vaug[t + kc][:, 1:2, :],
                                    start=(kc == 0), stop=(kc == 1),
                                )
                            # split the trailing normalize+DMA into halves
                            # on alternating engines so the last exposed
                            # piece is only 128 cols
                            nc.vector.tensor_scalar_mul(
                                ost[:, 256:384], paB[:, 0:128], rd[:])
                            nc.scalar.dma_start(
                                out=out_d[128 * t:128 * (t + 1), 256:384],
                                in_=ost[:, 256:384])
                            nc.vector.tensor_scalar_mul(
                                ost[:, 384:512], paB[:, 128:256], rd[:])
                            nc.sync.dma_start(
                                out=out_d[128 * t:128 * (t + 1), 384:512],
                                in_=ost[:, 384:512])

                # v-chunk projections interleave with attention pairs: the
                # v matmuls fill the PE while each pair's exp/mask/normalize
                # chain runs on the other engines
                v_chunks(0, 3)
                pair(0)
                v_chunks(3, 5)
                pair(1)
                v_chunks(5, 7)
                pair(2)
                v_chunks(7, 9)
                pair(3)
                v_chunks(9, 11)
                pair(4)
                v_chunks(11, 13)
                pair(5)
                v_chunks(13, 15)
                pair(6)
                v_chunks(15, 17)
                pair(7)
    nc.compile()
    return nc


def _get_nc():
    if "nc" not in _NC_CACHE:
        _NC_CACHE["nc"] = _build()
    return _NC_CACHE["nc"]


def _prep_shared(W_qkv, b_qkv, W_out, b_out):
    scale = 1.0 / np.sqrt(np.float64(E))
    W = np.array(W_qkv, dtype=np.float64)
    Wo = np.array(W_out, dtype=np.float64)
    b = np.array(b_qkv, dtype=np.float64)
    bo = np.array(b_out, dtype=np.float64)

    wq = W[:, :E] * scale
    wk = W[:, E:2 * E]
    wvo = W[:, 2 * E:3 * E] @ Wo          # fold output projection into v
    wqkv = np.concatenate([wq, wk, wvo], axis=1)

    bq = b[:E] * scale
    bk = b[E:2 * E]
    bqk = np.stack([*(bq.reshape(4, 128)), *(bk.reshape(4, 128))], axis=1)
    bvo = b[2 * E:3 * E] @ Wo + bo        # folded output bias

    # e-interleave: [512, 1536] -> [128, 4, 1536] with (p, e, c) = row 128e+p
    wqkv_i = np.ascontiguousarray(
        wqkv.reshape(4, 128, 3 * E).transpose(1, 0, 2))
    shared = {
        "wqkv": wqkv_i.astype(np.float32).astype(BF16_NP),
        "bqk": np.ascontiguousarray(bqk.astype(np.float32)),
        "brep": np.ascontiguousarray(
            np.tile(bvo.astype(np.float32)[None, :], (128, 1))),
    }
    return shared


def _masks_for(h: int) -> np.ndarray:
    """Multiplicative 0/1 masks in TRANSPOSED [key-in-chunk, block, query]
    layout for PAIRED tiles. Blocks per pair: [c0 | c1(for t0) | c1(for t1)
    | c2]; c0/c1(t0) see the key chunk as window-low (upper-triangular),
    c1(t1)/c2 as window-high (lower-triangular). Variants along dim1:
    [interior | first-pair | last-pair]."""
    j = np.arange(128)[:, None]           # key index within chunk
    i = np.arange(128)[None, :]           # query index within tile
    ut = (j - i >= 0)                     # key chunk == query tile: jj-i in [0,128]
    lt = (j <= i)                         # key chunk one above: jj-i in [0,128]
    ut_e = ut & (j >= 64) if h == 0 else ut       # seq start: key >= 0
    lt_e = lt & (j < 64) if h == 1 else lt        # seq end: key < S
    interior = np.stack([ut, lt, ut, lt], axis=1)
    first = np.stack([ut_e, lt, ut, lt], axis=1)
    last = np.stack([ut, lt, ut, lt_e], axis=1)
    stacked = np.concatenate([interior, first, last], axis=1)  # [128, 12, 128]
    return np.ascontiguousarray(
        stacked.reshape(128, 1536).astype(np.float32)).astype(BF16_NP)


def _install_ntff_shim():
    """The agent image's antenv lacks axon_hooks; synthesize it from the
    boot module's ctypes NTFF driver so trace=True can capture HW timing."""
    import types
    if "antenv.axon_hooks" in sys.modules:
        return
    try:
        from trn_agent_boot.trn_boot import _ntff_profile_via_ctypes
        hook = _ntff_profile_via_ctypes("/opt/axon/libaxon_pjrt.so")
    except Exception:
        hook = None
    mod = types.ModuleType("antenv.axon_hooks")
    mod.get_axon_ntff_profile_hook = lambda: hook
    mod.set_axon_ntff_profile_hook = lambda h: None
    sys.modules["antenv.axon_hooks"] = mod
    # avoid S3 artifact upload attempts during local profile processing
    try:
        from concourse import bass_utils as _bu
        _bu.upload_artifacts = lambda tmpdir: tmpdir
    except Exception:
        pass


def kernel(x, W_qkv, b_qkv, W_out, b_out, _trace=False):
    x = np.asarray(x, dtype=np.float32)
    nc = _get_nc()
    shared = _prep_shared(W_qkv, b_qkv, W_out, b_out)
    masks = [_masks_for(0), _masks_for(1)]

    in_maps = []
    for core in range(8):
        b, h = divmod(core, 2)
        lo = h * HALF - WINDOW
        hi = lo + ROWS
        xh = np.zeros((ROWS, E), dtype=np.float32)
        s0, s1 = max(lo, 0), min(hi, S)
        xh[s0 - lo:s1 - lo] = x[b, s0:s1]
        # transpose then e-interleave: [512, ROWS] -> [128, 4, ROWS]
        xT_i = np.ascontiguousarray(
            xh.T.reshape(4, 128, ROWS).transpose(1, 0, 2))
        in_maps.append({
            "xT": xT_i.astype(BF16_NP),
            "masks": masks[h],
            **shared,
        })

    kwargs = {}
    if _trace:
        _install_ntff_shim()
        kwargs = dict(trace=True, trace_cores=[0])
    res = run_bass_kernel_spmd(nc, in_maps, core_ids=list(range(8)), **kwargs)

    out = np.empty((B, S, E), dtype=np.float32)
    for core in range(8):
        b, h = divmod(core, 2)
        out[b, h * HALF:(h + 1) * HALF] = res.results[core]["out"].astype(np.float32)
    if _trace:
        return out, res
    return out

